# revision 1
# baseline (speedup 1.0000x reference)
"""Trainium2 Bass kernel for causal multi-head attention block.

Reference computation (fp32):
    qkv = x @ w_qkv;  q,k,v = split(qkv)
    attn = softmax(causal_mask(q k^T / sqrt(64)))
    out  = (attn @ v reassembled) @ w_out

Sharding over 8 NeuronCores: core c handles batch b = c//4 and heads
4*(c%4) .. 4*(c%4)+3 (4 of 16 heads).  Each core computes the rank-256
partial product of the output projection restricted to its heads'
channels; the host sums the 4 partials per batch.

Everything streams and computes in fp16 (fp32 PSUM accumulation),
halving HBM traffic versus fp32: inputs arrive pre-packed by the host in
the exact SBUF layout so every DMA line is 4-8KB contiguous, with x on
the sync queue and q/k weights on the scalar queue in parallel.  A
~3.4us warm-up matmul burst at t=0 releases the HAM clock gate (1.2 ->
2.4 GHz) before the first projection chains issue.  The attention inner
loops (q k^T, P v) pack 2 heads per PE pass via K=64 row groups; softmax
skips the max-subtraction (logits are O(10), fp32 exp is safe);
denominators ride along as a fused 65th lhsT column, and the reciprocal
runs on a [128,8] reshape via a DRAM round trip with a
partition-broadcast DMA.  Output is written fp16; the host sums the 4
partials per batch in fp32.  Scale-relative max err ~4.6e-4 vs the fp32
reference.

Emission order interleaves projection and attention at section
granularity -- ph1(0), ph1(1), attn(1), ph1(2), attn(0), ph1(3),
attn(2), proj(1), attn(3), proj(0), proj(2), proj(3) -- so the ScalarE
exp stream (the attention-phase rate limiter, ~81us/core) starts as
soon as the first two q/k blocks exist and later projection chains
overlap the exp backlog.  All PSUM pools coexist: qkv ring (2 banks) +
S/proj ring (4) + O accumulator (2).
"""

import sys

for _p in ("/opt/trn_rl_repo", "/root/.axon_site/_ro/trn_rl_repo"):
    if _p not in sys.path:
        sys.path.append(_p)

import numpy as np

import concourse.bass as bass
import concourse.mybir as mybir
import concourse.tile as tile
from concourse import bacc, bass_utils

P = 128
B, T, C = 2, 2048, 1024
HPC = 4            # heads per core
DH = 64            # head dim
KT = C // P        # 8 contraction tiles over d_model
NQB = T // 512     # 4 query blocks of 512
NKT = T // P       # 16 key tiles of 128
F32 = mybir.dt.float32
R32 = mybir.dt.float32r
F16 = mybir.dt.float16
EXP = mybir.ActivationFunctionType.Exp
SCALE = 1.0 / 8.0  # 1/sqrt(DH)


def _body(tc, nc, xT, wq, wk, wv, wo, tri, vones, out):
    # PE warm-up: ~3.4us of back-to-back matmuls on a zero tile so the HAM
    # clock gate releases (1.2 -> 2.4 GHz) before the real phase-1 chains.
    with (
        tc.tile_pool(name="wrm", bufs=1) as wsb,
        tc.tile_pool(name="wrmp", bufs=1, space="PSUM") as wpp,
    ):
        wsrc = wsb.tile([P, 64], F16, name="wsrc")
        nc.vector.memset(wsrc, 0.0)
        wdst = wpp.tile([64, 64], F32, name="wdst")
        NWARM = 64
        for i in range(NWARM):
            nc.tensor.matmul(
                wdst, wsrc, wsrc, start=(i == 0), stop=(i == NWARM - 1)
            )

    with tc.tile_pool(name="const", bufs=1) as cpool:
        wq_sb = cpool.tile([P, KT, 2 * P], F16, name="wq_sb")
        wk_sb = cpool.tile([P, KT, 2 * P], F16, name="wk_sb")
        wv_sb = cpool.tile([P, KT, 2 * P], F16, name="wv_sb")
        wo_sb = cpool.tile([P, 2, C], F16, name="wo_sb")
        tri_sb = cpool.tile([P, P], F16, name="tri_sb")
        # halves, interleaved with the first x tile, so the first
        # accumulation chain starts early; bulky later-phase constants go
        # through the gpsimd (SWDGE) queue so they don't delay the critical
        # path.
        nc.scalar.dma_start(wq_sb, wq)
        nc.scalar.dma_start(wk_sb, wk)
        nc.gpsimd.dma_start(wo_sb, wo)
        nc.gpsimd.dma_start(tri_sb, tri)

        # preload the exp ACT table set during the startup DMA window
        warm = cpool.tile([1, 2], F32, name="warm")
        nc.vector.memset(warm, 0.0)
        nc.scalar.activation(warm, warm, EXP, scale=1.0)

        # persistent stores
        qT = [cpool.tile([P, T], F16, name=f"qT{pr}") for pr in range(2)]
        kT = [cpool.tile([P, T], F16, name=f"kT{pr}") for pr in range(2)]
        # v with a fused ones column: [T-part, ktile, head, 65]
        vS = cpool.tile([P, NKT, HPC, DH + 1], F16, name="vS")
        nc.gpsimd.dma_start(vS[:, :, :, DH : DH + 1], vones)
        oT = [cpool.tile([P, T], F16, name=f"oT{pr}") for pr in range(2)]
        oTu = [cpool.tile([DH + 1, 2, T], F32, name=f"oTu{pr}") for pr in range(2)]

        # ---------- fused schedule: shared pools ----------
        # 8 PSUM banks: qkv ring (2) + sp ring (4, incl. proj pp) + op (2)
        with (
            tc.tile_pool(name="xt", bufs=1) as xpool,
            tc.tile_pool(name="psA", bufs=2, space="PSUM") as psA,
            tc.tile_pool(name="sps", bufs=2, space="PSUM") as sps,
            tc.tile_pool(name="ops", bufs=1, space="PSUM") as ops,
            tc.tile_pool(name="ptp", bufs=5) as ptp,
            tc.tile_pool(name="nrm", bufs=2) as nrm,
            tc.tile_pool(name="dsc", bufs=2, space="DRAM") as dsc,
            tc.tile_pool(name="osb", bufs=3) as osb,
        ):
            xts = [
                xpool.tile([P, KT, 512], F16, name=f"xt{i}", bufs=1)
                for i in range(NQB)
            ]
            nc.sync.dma_start(xts[0][:, 0:4, :], xT[:, 0, 0:4])
            nc.sync.dma_start(xts[0][:, 4:8, :], xT[:, 0, 4:8])
            nc.sync.dma_start(wv_sb, wv)
            for later in range(1, NQB):
                nc.sync.dma_start(xts[later], xT[:, later])

            def ph1_block(tb5):
                xt = xts[tb5]
                for w_sb, dst in ((wq_sb, qT), (wk_sb, kT)):
                    for pr in range(2):
                        qp = psA.tile([P, 512], F32, name="qp", tag="qkv")
                        for kt in range(KT):
                            nc.tensor.matmul(
                                qp,
                                w_sb[:, kt, pr * P : (pr + 1) * P],
                                xt[:, kt, :],
                                start=(kt == 0),
                                stop=(kt == KT - 1),
                            )
                        nc.vector.tensor_copy(
                            dst[pr][:, tb5 * 512 : (tb5 + 1) * 512], qp
                        )
                for sub in range(4):
                    tb1 = tb5 * 4 + sub
                    vp = psA.tile([P, 512], F32, name="vp", tag="qkv")
                    for kt in range(KT):
                        nc.tensor.matmul(
                            vp[:, 0:256],
                            xt[:, kt, sub * P : (sub + 1) * P],
                            wv_sb[:, kt, :],
                            start=(kt == 0),
                            stop=(kt == KT - 1),
                        )
                    nc.vector.tensor_copy(
                        vS[:, tb1, :, 0:DH],
                        vp[:, 0:256].rearrange("p (h d) -> p h d", d=DH),
                    )

            # S^T blocks: [k-tile(128) x q-block(<=512)] per head; 2 heads
            # packed on PE row groups (K=64 each).  exp via ScalarE.
            def emit_proj_one(tb1, tag, use_act=False):
                # output projection for one T block (both pairs' oT final).
                pp = sps.tile([P, 1024], F32, name="pp", tag=tag)
                for pr in range(2):
                    for cb in range(2):
                        nc.tensor.matmul(
                            pp[:, cb * 512 : (cb + 1) * 512],
                            oT[pr][:, tb1 * P : (tb1 + 1) * P],
                            wo_sb[:, pr, cb * 512 : (cb + 1) * 512],
                            start=(pr == 0),
                            stop=(pr == 1),
                        )
                ot = osb.tile([P, 1024], F16, name="ot")
                if use_act:
                    nc.scalar.copy(ot[:, 0:512], pp[:, 0:512])
                else:
                    nc.vector.tensor_copy(ot[:, 0:512], pp[:, 0:512])
                nc.vector.tensor_copy(ot[:, 512:1024], pp[:, 512:1024])
                nc.sync.dma_start(out[tb1 * P : (tb1 + 1) * P, :], ot)

            def attn_block(qb):
                for pr in range(2):
                    op = ops.tile([P, 1024], F32, name="op", tag="op")
                    nk = 4 * qb + 4

                    def geom(j):
                        r = j - 4 * qb
                        width = 512 - r * P if r >= 0 else 512
                        col0 = r * P if r >= 0 else 0
                        return r, width, col0

                    def emit_o(j, pts):
                        _, width, col0 = geom(j)
                        pt = pts.pop(j)
                        for h in range(2):
                            nc.tensor.matmul(
                                op[0 : DH + 1, h * 512 + col0 : (h + 1) * 512],
                                vS[:, j, pr * 2 + h, :],
                                pt[:, h * 512 : h * 512 + width],
                                start=(j == 0),
                                stop=(j == nk - 1),
                                skip_group_check=True,
                            )

                    pts = {}
                    for j in range(nk):
                        r, width, col0 = geom(j)
                        qoff = qb * 512 + col0
                        sp_ = sps.tile([P, 1024], F32, name="sp_", tag="sp")
                        for h in range(2):
                            nc.tensor.matmul(
                                sp_[:, h * 512 : h * 512 + width],
                                kT[pr][h * DH : (h + 1) * DH, j * P : (j + 1) * P],
                                qT[pr][h * DH : (h + 1) * DH, qoff : qoff + width],
                                start=True,
                                stop=True,
                            )
                        pt = ptp.tile([P, 1024], F16, name="pt")
                        s3 = sp_.rearrange("p (h w) -> p h w", h=2)[:, :, 0:width]
                        p3 = pt.rearrange("p (h w) -> p h w", h=2)[:, :, 0:width]
                        nc.scalar.activation(p3, s3, EXP, scale=SCALE)
                        if r >= 0:
                            # triangular mask on the first 128 valid columns
                            for h in range(2):
                                nc.vector.tensor_mul(
                                    pt[:, h * 512 : h * 512 + P],
                                    pt[:, h * 512 : h * 512 + P],
                                    tri_sb,
                                )
                        pts[j] = pt
                        # software pipeline: O' lagged two steps behind S/exp
                        if j > 1:
                            emit_o(j - 2, pts)
                    if nk > 1:
                        emit_o(nk - 2, pts)
                    emit_o(nk - 1, pts)

                    # fast unnormalized evacuation: frees the op PSUM slot
                    # ~1.4us after the chain; the reciprocal/broadcast chain
                    # then runs against SBUF off the critical resource.
                    qs = slice(qb * 512, (qb + 1) * 512)
                    nc.vector.tensor_copy(
                        oTu[pr][:, :, qs],
                        op.rearrange("p (h w) -> p h w", h=2)[0 : DH + 1],
                    )
                    dd = dsc.tile([1024], F32, name="dd", tag="dd")
                    nc.sync.dma_start(
                        dd.rearrange("(h w) -> h w", h=2)[None],
                        oTu[pr][DH : DH + 1, :, qs],
                    )
                    rsh = nrm.tile([P, 8], F32, name="rsh", tag="rsh")
                    nc.sync.dma_start(rsh, dd.rearrange("(p c) -> p c", p=P))
                    rr = nrm.tile([P, 8], F32, name="rr", tag="rr")
                    nc.vector.reciprocal(rr, rsh)
                    dd2 = dsc.tile([1024], F32, name="dd2", tag="dd2")
                    nc.sync.dma_start(dd2.rearrange("(p c) -> p c", p=P), rr)
                    bc = nrm.tile([DH, 1024], F32, name="bc", tag="bc")
                    nc.sync.dma_start(
                        bc[:, 0:512], dd2[None, 0:512].to_broadcast([DH, 512])
                    )
                    nc.sync.dma_start(
                        bc[:, 512:1024], dd2[None, 512:1024].to_broadcast([DH, 512])
                    )
                    nc.vector.tensor_mul(
                        oT[pr][0:DH, qs], oTu[pr][0:DH, 0, qs], bc[:, 0:512]
                    )
                    o1 = nrm.tile([DH, 512], F16, name="o1", tag="o1")
                    nc.vector.tensor_mul(o1, oTu[pr][0:DH, 1, qs], bc[:, 512:1024])
                    nc.sync.dma_start(oT[pr][DH : 2 * DH, qs], o1)

            # section-interleaved schedule: each attention block is emitted
            # as soon as its q/k chains exist, so the ScalarE exp stream
            # starts ~25us in (instead of after all of phase 1) and later
            # projection blocks overlap the exp backlog.  Projections lag
            # two sections behind their normalization chains.
            ph1_block(0)
            ph1_block(1)
            attn_block(1)
            ph1_block(2)
            attn_block(0)
            ph1_block(3)
            attn_block(2)
            for tb1 in range(4, 8):
                emit_proj_one(tb1, "sp")
            attn_block(3)
            for tb1 in range(0, 4):
                emit_proj_one(tb1, "sp")
            for pq in (2, 3):
                for tb1 in range(pq * 4, pq * 4 + 4):
                    emit_proj_one(tb1, "sp", use_act=True)


def build_bass():
    nc = bacc.Bacc("TRN2", target_bir_lowering=False, debug=False, num_devices=8)
    xT = nc.dram_tensor("xT", [P, NQB, KT, 512], F16, kind="ExternalInput").ap()
    wq = nc.dram_tensor("wq", [P, KT, 2 * P], F16, kind="ExternalInput").ap()
    wk = nc.dram_tensor("wk", [P, KT, 2 * P], F16, kind="ExternalInput").ap()
    wv = nc.dram_tensor("wv", [P, KT, 2 * P], F16, kind="ExternalInput").ap()
    wo = nc.dram_tensor("wo", [P, 2, C], F16, kind="ExternalInput").ap()
    tri = nc.dram_tensor("tri", [P, P], F16, kind="ExternalInput").ap()
    vones = nc.dram_tensor(
        "vones", [P, NKT, HPC, 1], F16, kind="ExternalInput"
    ).ap()
    out = nc.dram_tensor("out", [T, C], F16, kind="ExternalOutput").ap()
    with tile.TileContext(nc) as tc:
        _body(tc, nc, xT, wq, wk, wv, wo, tri, vones, out)
    nc.compile()
    return nc


def _fp32r(a):
    """Round fp32 to fp32r format: 11-bit mantissa, low 12 bits zero (RTNE)."""
    u = np.ascontiguousarray(a, dtype=np.float32).view(np.uint32)
    r = (u + 0x7FF + ((u >> 12) & 1)) & np.uint32(0xFFFFF000)
    return np.ascontiguousarray(r.view(np.float32))


def make_in_maps(x, w_qkv, w_out):
    """Host-side sharding: returns the 8 per-core input dicts."""
    x = np.ascontiguousarray(np.asarray(x, dtype=np.float32))
    w_qkv = np.ascontiguousarray(np.asarray(w_qkv, dtype=np.float32))
    w_out = np.ascontiguousarray(np.asarray(w_out, dtype=np.float32))
    kk = np.arange(P)
    tri = (kk[None, :] >= kk[:, None]).astype(np.float16)  # [k, q]: q >= k

    def pack_w(w):  # [C, n] -> [P, KT, n], per-partition contiguous
        n = w.shape[1]
        return np.ascontiguousarray(
            w.astype(np.float16).reshape(KT, P, n).transpose(1, 0, 2)
        )

    # x[b].T [C, T] -> [P, block, kt, 512], fully contiguous per partition
    xTb = [
        np.ascontiguousarray(
            x[b].T.astype(np.float16)
            .reshape(KT, P, NQB, 512)
            .transpose(1, 2, 0, 3)
        )
        for b in range(B)
    ]
    in_maps = []
    for c in range(8):
        b = c // 4
        g = c % 4
        h0 = HPC * g * DH  # 256*g
        wo_c = w_out[h0 : h0 + 2 * P, :].astype(np.float16)
        in_maps.append(
            {
                "xT": xTb[b],
                "wq": pack_w(w_qkv[:, h0 : h0 + 2 * P]),
                "wk": pack_w(w_qkv[:, C + h0 : C + h0 + 2 * P]),
                "wv": pack_w(w_qkv[:, 2 * C + h0 : 2 * C + h0 + 2 * P]),
                "wo": np.ascontiguousarray(
                    wo_c.reshape(2, P, C).transpose(1, 0, 2)
                ),
                "tri": np.ascontiguousarray(tri),
                "vones": np.ones((P, NKT, HPC, 1), dtype=np.float16),
            }
        )
    return in_maps


_NC_CACHE = None
LAST_RESULTS = None  # BassKernelResults of the most recent run (for profiling)
TRACE = False


def kernel(x, w_qkv, w_out):
    global _NC_CACHE, LAST_RESULTS
    if _NC_CACHE is None:
        _NC_CACHE = build_bass()
    nc = _NC_CACHE
    in_maps = make_in_maps(x, w_qkv, w_out)
    res = bass_utils.run_bass_kernel_spmd(
        nc, in_maps, core_ids=list(range(8)), trace=TRACE
    )
    LAST_RESULTS = res
    partials = [res.results[c]["out"] for c in range(8)]
    out = np.zeros((B, T, C), dtype=np.float32)
    for c in range(8):
        out[c // 4] += partials[c].astype(np.float32)
    return out


if __name__ == "__main__":
    # smoke test with random data
    rng = np.random.default_rng(0)
    x = rng.standard_normal((B, T, C), dtype=np.float32)
    w_qkv = rng.standard_normal((C, 3 * C), dtype=np.float32) / np.sqrt(C)
    w_out = rng.standard_normal((C, C), dtype=np.float32) / np.sqrt(C)
    o = kernel(x, w_qkv, w_out)
    print(o.shape, o.dtype)



# revision 4
# speedup vs baseline: 1.0578x; 1.0578x over previous
"""Trainium2 Bass kernel for causal multi-head attention block.

Reference computation (fp32):
    qkv = x @ w_qkv;  q,k,v = split(qkv)
    attn = softmax(causal_mask(q k^T / sqrt(64)))
    out  = (attn @ v reassembled) @ w_out

Sharding over 8 NeuronCores: core c handles batch b = c//4 and heads
4*(c%4) .. 4*(c%4)+3 (4 of 16 heads).  Each core computes the rank-256
partial product of the output projection restricted to its heads'
channels; the host sums the 4 partials per batch.

Everything streams and computes in fp16 (fp32 PSUM accumulation).
The warm-up matmul burst and all constant tiles live in pools that stay
allocated for the whole kernel, so no SBUF address is reused between the
warm-up and the weight tiles -- input DMAs start at ~6.5us (right after
the NEFF preamble) instead of waiting on the warm-up semaphore.

Schedule: a single software-pipelined stream.  Each attention section
(qb, pr) runs the baseline lag-2 S/exp/PV pipeline and additionally
interleaves "filler" chunks (q/k projection chains of later blocks, v
blocks, output-projection tiles of earlier blocks) into its j loop so
the PE stays dense while the ScalarE exp stream paces ~0.6us/j behind.
"""

import sys

for _p in ("/opt/trn_rl_repo", "/root/.axon_site/_ro/trn_rl_repo"):
    if _p not in sys.path:
        sys.path.append(_p)

import numpy as np

import concourse.bass as bass
import concourse.mybir as mybir
import concourse.tile as tile
from concourse import bacc, bass_utils

P = 128
B, T, C = 2, 2048, 1024
HPC = 4            # heads per core
DH = 64            # head dim
KT = C // P        # 8 contraction tiles over d_model
NQB = T // 512     # 4 query blocks of 512
NKT = T // P       # 16 key tiles of 128
F32 = mybir.dt.float32
F16 = mybir.dt.float16
EXP = mybir.ActivationFunctionType.Exp
SCALE = 1.0 / 8.0  # 1/sqrt(DH)
NWARM = 48


def _body(tc, nc, xT, wq, wk, wv, wo, tri, vones, out):
    with (
        tc.tile_pool(name="const", bufs=1) as cpool,
        tc.tile_pool(name="xt", bufs=1) as xpool,
        tc.tile_pool(name="psA", bufs=2, space="PSUM") as psA,
        tc.tile_pool(name="sps", bufs=2, space="PSUM") as sps,
        tc.tile_pool(name="ops", bufs=1, space="PSUM") as ops,
        tc.tile_pool(name="ptp", bufs=8) as ptp,
        tc.tile_pool(name="nrm", bufs=2) as nrm,
        tc.tile_pool(name="dsc", bufs=2, space="DRAM") as dsc,
        tc.tile_pool(name="osb", bufs=3) as osb,
    ):
        # ---- PE warm-up: releases the HAM clock gate (1.2 -> 2.4 GHz)
        # during the NEFF preamble + input-DMA window.  The source tile
        # lives in the persistent const pool and the PSUM target shares
        # the "op" ring slot, so no SBUF/PSUM address is ever handed back
        # for reuse -- the input DMAs don't wait on the warm-up.
        wsrc = cpool.tile([P, 64], F16, name="wsrc")
        nc.vector.memset(wsrc, 0.0)
        wdst = ops.tile([64, 64], F32, name="wdst", tag="op")
        for i in range(NWARM):
            nc.tensor.matmul(
                wdst, wsrc, wsrc, start=(i == 0), stop=(i == NWARM - 1)
            )

        # ---- constants.  wq/wk are packed pr-major so each 256KB half is
        # one contiguous DMA and the first q/k chains only wait on their
        # own half.
        wq_sb = cpool.tile([P, 2, KT, P], F16, name="wq_sb")
        wk_sb = cpool.tile([P, 2, KT, P], F16, name="wk_sb")
        wv_sb = cpool.tile([P, KT, 2 * P], F16, name="wv_sb")
        wo_sb = cpool.tile([P, 2, C], F16, name="wo_sb")
        tri_sb = cpool.tile([P, P], F16, name="tri_sb")
        nc.scalar.dma_start(wq_sb[:, 0], wq[:, 0])
        nc.scalar.dma_start(wk_sb[:, 0], wk[:, 0])
        nc.scalar.dma_start(wq_sb[:, 1], wq[:, 1])
        nc.scalar.dma_start(wk_sb[:, 1], wk[:, 1])
        nc.gpsimd.dma_start(wo_sb, wo)
        nc.gpsimd.dma_start(tri_sb, tri)

        # preload the exp ACT table set during the startup DMA window
        warm = cpool.tile([1, 2], F32, name="warm")
        nc.vector.memset(warm, 0.0)
        nc.scalar.activation(warm, warm, EXP, scale=1.0)

        # persistent stores
        qT = [cpool.tile([P, T], F16, name=f"qT{pr}") for pr in range(2)]
        kT = [cpool.tile([P, T], F16, name=f"kT{pr}") for pr in range(2)]
        # v with a fused ones column: [T-part, ktile, head, 65]
        vS = cpool.tile([P, NKT, HPC, DH + 1], F16, name="vS")
        nc.gpsimd.dma_start(vS[:, :, :, DH : DH + 1], vones)
        oT = [cpool.tile([P, T], F16, name=f"oT{pr}") for pr in range(2)]
        oTu = [cpool.tile([DH + 1, 2, T], F32, name=f"oTu{pr}") for pr in range(2)]

        xts = [
            xpool.tile([P, KT, 512], F16, name=f"xt{i}", bufs=1)
            for i in range(NQB)
        ]
        nc.sync.dma_start(xts[0][:, 0:4, :], xT[:, 0, 0:4])
        nc.sync.dma_start(xts[0][:, 4:8, :], xT[:, 0, 4:8])
        nc.sync.dma_start(wv_sb, wv)
        for later in range(1, NQB):
            nc.sync.dma_start(xts[later], xT[:, later])

        # ---------- building blocks ----------
        def qk_chain(tb5, pr, w_sb, dst):
            xt = xts[tb5]
            qp = psA.tile([P, 512], F32, name="qp", tag="qkv")
            for kt in range(KT):
                nc.tensor.matmul(
                    qp,
                    w_sb[:, pr, kt, :],
                    xt[:, kt, :],
                    start=(kt == 0),
                    stop=(kt == KT - 1),
                )
            nc.vector.tensor_copy(dst[pr][:, tb5 * 512 : (tb5 + 1) * 512], qp)

        def v_sub(tb5, sub):
            xt = xts[tb5]
            tb1 = tb5 * 4 + sub
            vp = psA.tile([P, 512], F32, name="vp", tag="qkv")
            for kt in range(KT):
                nc.tensor.matmul(
                    vp[:, 0:256],
                    xt[:, kt, sub * P : (sub + 1) * P],
                    wv_sb[:, kt, :],
                    start=(kt == 0),
                    stop=(kt == KT - 1),
                )
            nc.vector.tensor_copy(
                vS[:, tb1, :, 0:DH],
                vp[:, 0:256].rearrange("p (h d) -> p h d", d=DH),
            )

        def emit_proj_one(tb1, use_act=False):
            # output projection for one 128-token block.
            pp = sps.tile([P, 1024], F32, name="pp", tag="sp")
            for pr in range(2):
                for cb in range(2):
                    nc.tensor.matmul(
                        pp[:, cb * 512 : (cb + 1) * 512],
                        oT[pr][:, tb1 * P : (tb1 + 1) * P],
                        wo_sb[:, pr, cb * 512 : (cb + 1) * 512],
                        start=(pr == 0),
                        stop=(pr == 1),
                    )
            ot = osb.tile([P, 1024], F16, name="ot")
            if use_act:
                nc.scalar.copy(ot[:, 0:512], pp[:, 0:512])
            else:
                nc.vector.tensor_copy(ot[:, 0:512], pp[:, 0:512])
            nc.vector.tensor_copy(ot[:, 512:1024], pp[:, 512:1024])
            nc.sync.dma_start(out[tb1 * P : (tb1 + 1) * P, :], ot)

        def attn_block(qb, pr, fillers=()):
            """S^T blocks [k-tile(128) x q-block(<=512)]; 2 heads packed on
            PE row groups (K=64 each); exp via ScalarE; PV lagged 2 steps;
            `fillers` (thunks of ~0.8-1.8us PE work) are drained evenly
            across the j loop to keep the PE dense while exp paces."""
            fillers = list(fillers)
            op = ops.tile([P, 1024], F32, name="op", tag="op")
            nk = 4 * qb + 4

            def geom(j):
                r = j - 4 * qb
                width = 512 - r * P if r >= 0 else 512
                col0 = r * P if r >= 0 else 0
                return r, width, col0

            def emit_o(j, pts):
                _, width, col0 = geom(j)
                pt = pts.pop(j)
                for h in range(2):
                    nc.tensor.matmul(
                        op[0 : DH + 1, h * 512 + col0 : (h + 1) * 512],
                        vS[:, j, pr * 2 + h, :],
                        pt[:, h * 512 : h * 512 + width],
                        start=(j == 0),
                        stop=(j == nk - 1),
                        skip_group_check=True,
                    )

            pts = {}
            nf = len(fillers)
            fdone = 0
            for j in range(nk):
                r, width, col0 = geom(j)
                qoff = qb * 512 + col0
                sp_ = sps.tile([P, 1024], F32, name="sp_", tag="sp")
                for h in range(2):
                    nc.tensor.matmul(
                        sp_[:, h * 512 : h * 512 + width],
                        kT[pr][h * DH : (h + 1) * DH, j * P : (j + 1) * P],
                        qT[pr][h * DH : (h + 1) * DH, qoff : qoff + width],
                        start=True,
                        stop=True,
                    )
                pt = ptp.tile([P, 1024], F16, name="pt")
                s3 = sp_.rearrange("p (h w) -> p h w", h=2)[:, :, 0:width]
                p3 = pt.rearrange("p (h w) -> p h w", h=2)[:, :, 0:width]
                nc.scalar.activation(p3, s3, EXP, scale=SCALE)
                if r >= 0:
                    # triangular mask on the first 128 valid columns
                    for h in range(2):
                        nc.vector.tensor_mul(
                            pt[:, h * 512 : h * 512 + P],
                            pt[:, h * 512 : h * 512 + P],
                            tri_sb,
                        )
                pts[j] = pt
                # spread fillers evenly over the j loop
                want = nf * (j + 1) // nk
                while fdone < want:
                    fillers[fdone]()
                    fdone += 1
                # software pipeline: O' lagged two steps behind S/exp
                if j > 1:
                    emit_o(j - 2, pts)
            if nk > 1:
                emit_o(nk - 2, pts)
            emit_o(nk - 1, pts)
            while fdone < nf:
                fillers[fdone]()
                fdone += 1

            # fast unnormalized evacuation frees the op PSUM slot; the
            # reciprocal/broadcast chain then runs off the critical engine.
            qs = slice(qb * 512, (qb + 1) * 512)
            nc.vector.tensor_copy(
                oTu[pr][:, :, qs],
                op.rearrange("p (h w) -> p h w", h=2)[0 : DH + 1],
            )
            dd = dsc.tile([1024], F32, name="dd", tag="dd")
            nc.sync.dma_start(
                dd.rearrange("(h w) -> h w", h=2)[None],
                oTu[pr][DH : DH + 1, :, qs],
            )
            rsh = nrm.tile([P, 8], F32, name="rsh", tag="rsh")
            nc.sync.dma_start(rsh, dd.rearrange("(p c) -> p c", p=P))
            rr = nrm.tile([P, 8], F32, name="rr", tag="rr")
            nc.vector.reciprocal(rr, rsh)
            dd2 = dsc.tile([1024], F32, name="dd2", tag="dd2")
            nc.sync.dma_start(dd2.rearrange("(p c) -> p c", p=P), rr)
            bc = nrm.tile([DH, 1024], F32, name="bc", tag="bc")
            nc.sync.dma_start(
                bc[:, 0:512], dd2[None, 0:512].to_broadcast([DH, 512])
            )
            nc.sync.dma_start(
                bc[:, 512:1024], dd2[None, 512:1024].to_broadcast([DH, 512])
            )
            nc.vector.tensor_mul(
                oT[pr][0:DH, qs], oTu[pr][0:DH, 0, qs], bc[:, 0:512]
            )
            o1 = nrm.tile([DH, 512], F16, name="o1", tag="o1")
            nc.vector.tensor_mul(o1, oTu[pr][0:DH, 1, qs], bc[:, 512:1024])
            nc.sync.dma_start(oT[pr][DH : 2 * DH, qs], o1)

        # ---------- schedule ----------
        # Leading q/k chains for block 0 (scalar-idle warm-up region),
        # then attention sections with fillers drawn from later blocks'
        # projections and earlier blocks' output projections.
        for pr in range(2):
            qk_chain(0, pr, wq_sb, qT)
            qk_chain(0, pr, wk_sb, kT)
        for sub in range(4):
            v_sub(0, sub)

        def F(fn, *a, **kw):
            return lambda: fn(*a, **kw)

        attn_block(0, 0, [F(qk_chain, 1, 0, wq_sb, qT), F(qk_chain, 1, 0, wk_sb, kT)])
        attn_block(0, 1, [F(qk_chain, 1, 1, wq_sb, qT), F(qk_chain, 1, 1, wk_sb, kT)])
        attn_block(1, 0, [F(v_sub, 1, 0), F(v_sub, 1, 1), F(v_sub, 1, 2),
                          F(v_sub, 1, 3), F(emit_proj_one, 0), F(emit_proj_one, 1)])
        attn_block(1, 1, [F(qk_chain, 2, 0, wq_sb, qT), F(qk_chain, 2, 0, wk_sb, kT),
                          F(emit_proj_one, 2), F(emit_proj_one, 3)])
        attn_block(2, 0, [F(qk_chain, 2, 1, wq_sb, qT), F(qk_chain, 2, 1, wk_sb, kT),
                          F(v_sub, 2, 0), F(v_sub, 2, 1), F(v_sub, 2, 2), F(v_sub, 2, 3)])
        attn_block(2, 1, [F(qk_chain, 3, 0, wq_sb, qT), F(qk_chain, 3, 0, wk_sb, kT),
                          F(qk_chain, 3, 1, wq_sb, qT), F(qk_chain, 3, 1, wk_sb, kT)])
        attn_block(3, 0, [F(v_sub, 3, 0), F(v_sub, 3, 1), F(v_sub, 3, 2), F(v_sub, 3, 3),
                          F(emit_proj_one, 4), F(emit_proj_one, 5),
                          F(emit_proj_one, 6), F(emit_proj_one, 7)])
        attn_block(3, 1, [F(emit_proj_one, 8), F(emit_proj_one, 9),
                          F(emit_proj_one, 10), F(emit_proj_one, 11)])
        for tb1 in range(12, 16):
            emit_proj_one(tb1, use_act=True)


def build_bass():
    nc = bacc.Bacc("TRN2", target_bir_lowering=False, debug=False, num_devices=8)
    xT = nc.dram_tensor("xT", [P, NQB, KT, 512], F16, kind="ExternalInput").ap()
    wq = nc.dram_tensor("wq", [P, 2, KT, P], F16, kind="ExternalInput").ap()
    wk = nc.dram_tensor("wk", [P, 2, KT, P], F16, kind="ExternalInput").ap()
    wv = nc.dram_tensor("wv", [P, KT, 2 * P], F16, kind="ExternalInput").ap()
    wo = nc.dram_tensor("wo", [P, 2, C], F16, kind="ExternalInput").ap()
    tri = nc.dram_tensor("tri", [P, P], F16, kind="ExternalInput").ap()
    vones = nc.dram_tensor(
        "vones", [P, NKT, HPC, 1], F16, kind="ExternalInput"
    ).ap()
    out = nc.dram_tensor("out", [T, C], F16, kind="ExternalOutput").ap()
    with tile.TileContext(nc) as tc:
        _body(tc, nc, xT, wq, wk, wv, wo, tri, vones, out)
    nc.compile()
    return nc


def make_in_maps(x, w_qkv, w_out):
    """Host-side sharding: returns the 8 per-core input dicts."""
    x = np.ascontiguousarray(np.asarray(x, dtype=np.float32))
    w_qkv = np.ascontiguousarray(np.asarray(w_qkv, dtype=np.float32))
    w_out = np.ascontiguousarray(np.asarray(w_out, dtype=np.float32))
    kk = np.arange(P)
    tri = (kk[None, :] >= kk[:, None]).astype(np.float16)  # [k, q]: q >= k

    def pack_w_pr(w):  # [C, 2P] -> [P, 2, KT, P], pr-major contiguous halves
        return np.ascontiguousarray(
            w.astype(np.float16).reshape(KT, P, 2, P).transpose(1, 2, 0, 3)
        )

    def pack_w(w):  # [C, n] -> [P, KT, n], per-partition contiguous
        n = w.shape[1]
        return np.ascontiguousarray(
            w.astype(np.float16).reshape(KT, P, n).transpose(1, 0, 2)
        )

    # x[b].T [C, T] -> [P, block, kt, 512], fully contiguous per partition
    xTb = [
        np.ascontiguousarray(
            x[b].T.astype(np.float16)
            .reshape(KT, P, NQB, 512)
            .transpose(1, 2, 0, 3)
        )
        for b in range(B)
    ]
    in_maps = []
    for c in range(8):
        b = c // 4
        g = c % 4
        h0 = HPC * g * DH  # 256*g
        wo_c = w_out[h0 : h0 + 2 * P, :].astype(np.float16)
        in_maps.append(
            {
                "xT": xTb[b],
                "wq": pack_w_pr(w_qkv[:, h0 : h0 + 2 * P]),
                "wk": pack_w_pr(w_qkv[:, C + h0 : C + h0 + 2 * P]),
                "wv": pack_w(w_qkv[:, 2 * C + h0 : 2 * C + h0 + 2 * P]),
                "wo": np.ascontiguousarray(
                    wo_c.reshape(2, P, C).transpose(1, 0, 2)
                ),
                "tri": np.ascontiguousarray(tri),
                "vones": np.ones((P, NKT, HPC, 1), dtype=np.float16),
            }
        )
    return in_maps


_NC_CACHE = None
LAST_RESULTS = None  # BassKernelResults of the most recent run (for profiling)
TRACE = False


def kernel(x, w_qkv, w_out):
    global _NC_CACHE, LAST_RESULTS
    if _NC_CACHE is None:
        _NC_CACHE = build_bass()
    nc = _NC_CACHE
    in_maps = make_in_maps(x, w_qkv, w_out)
    res = bass_utils.run_bass_kernel_spmd(
        nc, in_maps, core_ids=list(range(8)), trace=TRACE
    )
    LAST_RESULTS = res
    partials = [res.results[c]["out"] for c in range(8)]
    out = np.zeros((B, T, C), dtype=np.float32)
    for c in range(8):
        out[c // 4] += partials[c].astype(np.float32)
    return out


if __name__ == "__main__":
    # smoke test with random data
    rng = np.random.default_rng(0)
    x = rng.standard_normal((B, T, C), dtype=np.float32)
    w_qkv = rng.standard_normal((C, 3 * C), dtype=np.float32) / np.sqrt(C)
    w_out = rng.standard_normal((C, C), dtype=np.float32) / np.sqrt(C)
    o = kernel(x, w_qkv, w_out)
    print(o.shape, o.dtype)


# revision 12
# speedup vs baseline: 1.0964x; 1.0365x over previous
"""Trainium2 Bass kernel for causal multi-head attention block.

Reference computation (fp32):
    qkv = x @ w_qkv;  q,k,v = split(qkv)
    attn = softmax(causal_mask(q k^T / sqrt(64)))
    out  = (attn @ v reassembled) @ w_out

Sharding over 8 NeuronCores: core c handles batch b = c//4 and heads
4*(c%4) .. 4*(c%4)+3 (4 of 16 heads).  Each core computes the rank-256
partial product of the output projection restricted to its heads'
channels; the host sums the 4 partials per batch.

Everything streams and computes in fp16 (fp32 PSUM accumulation).
The warm-up matmul burst and all constant tiles live in pools that stay
allocated for the whole kernel, so no SBUF address is reused between the
warm-up and the weight tiles -- input DMAs start at ~6.5us (right after
the NEFF preamble) instead of waiting on the warm-up semaphore.

Schedule: a single software-pipelined stream.  Each attention section
(qb, pr) runs the baseline lag-2 S/exp/PV pipeline and additionally
interleaves "filler" chunks (q/k projection chains of later blocks, v
blocks, output-projection tiles of earlier blocks) into its j loop so
the PE stays dense while the ScalarE exp stream paces ~0.6us/j behind.
"""

import sys

for _p in ("/opt/trn_rl_repo", "/root/.axon_site/_ro/trn_rl_repo"):
    if _p not in sys.path:
        sys.path.append(_p)

import numpy as np

import concourse.bass as bass
import concourse.mybir as mybir
import concourse.tile as tile
from concourse import bacc, bass_utils

P = 128
B, T, C = 2, 2048, 1024
HPC = 4            # heads per core
DH = 64            # head dim
KT = C // P        # 8 contraction tiles over d_model
NQB = T // 512     # 4 query blocks of 512
NKT = T // P       # 16 key tiles of 128
F32 = mybir.dt.float32
R32 = mybir.dt.float32r
F16 = mybir.dt.float16
EXP = mybir.ActivationFunctionType.Exp
SCALE = 1.0 / 8.0  # 1/sqrt(DH)
# >= 3.4us of back-to-back matmuls at the cold 1.2 GHz clock (53ns per
# N=64 matmul), so the HAM activity window flips to K=8/8 before the
# first projection chains issue.  48 was not enough (2.5us) and the
# whole first phase ran at half clock.
NWARM = 80


def _body(tc, nc, xT, wq, wk, wv, wo, tri, vones, out):
    with (
        tc.tile_pool(name="const", bufs=1) as cpool,
        tc.tile_pool(name="xt", bufs=1) as xpool,
        tc.tile_pool(name="psA", bufs=2, space="PSUM") as psA,
        tc.tile_pool(name="sps", bufs=2, space="PSUM") as sps,
        tc.tile_pool(name="ops", bufs=1, space="PSUM") as ops,
        tc.tile_pool(name="ptp", bufs=8) as ptp,
        tc.tile_pool(name="nrm", bufs=2) as nrm,
        tc.tile_pool(name="dsc", bufs=2, space="DRAM") as dsc,
        tc.tile_pool(name="osb", bufs=3) as osb,
    ):
        # ---- PE warm-up: releases the HAM clock gate (1.2 -> 2.4 GHz)
        # during the NEFF preamble + input-DMA window.  The source tile
        # lives in the persistent const pool and the PSUM target shares
        # the "op" ring slot, so no SBUF/PSUM address is ever handed back
        # for reuse -- the input DMAs don't wait on the warm-up.
        wsrc = cpool.tile([P, 64], F16, name="wsrc")
        nc.vector.memset(wsrc, 0.0)
        wdst = ops.tile([64, 64], F32, name="wdst", tag="op")
        for i in range(NWARM):
            nc.tensor.matmul(
                wdst, wsrc, wsrc, start=(i == 0), stop=(i == NWARM - 1)
            )

        # ---- constants.  wq/wk are packed pr-major so each 256KB half is
        # one contiguous DMA and the first q/k chains only wait on their
        # own half.
        wq_sb = cpool.tile([P, 2, KT, P], F16, name="wq_sb")
        wk_sb = cpool.tile([P, 2, KT, P], F16, name="wk_sb")
        wv_sb = cpool.tile([P, KT, 2 * P], F16, name="wv_sb")
        wo_sb = cpool.tile([P, 2, C], F16, name="wo_sb")
        tri_sb = cpool.tile([P, P], F16, name="tri_sb")
        nc.scalar.dma_start(wq_sb[:, 0], wq[:, 0])
        nc.scalar.dma_start(wk_sb[:, 0], wk[:, 0])
        nc.scalar.dma_start(wq_sb[:, 1], wq[:, 1])
        nc.scalar.dma_start(wk_sb[:, 1], wk[:, 1])
        # tri (needed by the first attention section) and vones (first PV)
        # go ahead of the bulky wo (first needed by proj at ~60us) so the
        # SWDGE queue doesn't crowd the critical startup window.
        nc.gpsimd.dma_start(tri_sb, tri)

        # preload the exp ACT table set during the startup DMA window
        warm = cpool.tile([1, 2], F32, name="warm")
        nc.vector.memset(warm, 0.0)
        nc.scalar.activation(warm, warm, EXP, scale=1.0)

        # all-ones stationary column used to broadcast the softmax
        # denominators across partitions with a K=1 matmul (fast-path
        # normalize for the last attention section).
        ones_bc = cpool.tile([P, DH], F32, name="ones_bc")
        nc.vector.memset(ones_bc, 1.0)

        # persistent stores
        qT = [cpool.tile([P, T], F16, name=f"qT{pr}") for pr in range(2)]
        kT = [cpool.tile([P, T], F16, name=f"kT{pr}") for pr in range(2)]
        # v with a fused ones column: [T-part, ktile, head, 65]
        vS = cpool.tile([P, NKT, HPC, DH + 1], F16, name="vS")
        nc.gpsimd.dma_start(vS[:, :, :, DH : DH + 1], vones)
        oT = [cpool.tile([P, T], F16, name=f"oT{pr}") for pr in range(2)]
        oTu = [cpool.tile([DH + 1, 2, T], F32, name=f"oTu{pr}") for pr in range(2)]

        xts = [
            xpool.tile([P, KT, 512], F16, name=f"xt{i}", bufs=1)
            for i in range(NQB)
        ]
        for qu in range(4):
            nc.sync.dma_start(
                xts[0][:, 2 * qu : 2 * qu + 2, :], xT[:, 0, 2 * qu : 2 * qu + 2]
            )
        nc.sync.dma_start(wv_sb, wv)
        for later in range(1, NQB):
            nc.sync.dma_start(xts[later], xT[:, later])

        # ---------- building blocks ----------
        def qk_chain(tb5, pr, w_sb, dst):
            xt = xts[tb5]
            qp = psA.tile([P, 512], F32, name="qp", tag="qkv")
            for kt in range(KT):
                nc.tensor.matmul(
                    qp,
                    w_sb[:, pr, kt, :],
                    xt[:, kt, :],
                    start=(kt == 0),
                    stop=(kt == KT - 1),
                )
            nc.vector.tensor_copy(dst[pr][:, tb5 * 512 : (tb5 + 1) * 512], qp)

        def v_sub(tb5, sub):
            xt = xts[tb5]
            tb1 = tb5 * 4 + sub
            vp = psA.tile([P, 512], F32, name="vp", tag="qkv")
            for kt in range(KT):
                nc.tensor.matmul(
                    vp[:, 0:256],
                    xt[:, kt, sub * P : (sub + 1) * P],
                    wv_sb[:, kt, :],
                    start=(kt == 0),
                    stop=(kt == KT - 1),
                )
            nc.vector.tensor_copy(
                vS[:, tb1, :, 0:DH],
                vp[:, 0:256].rearrange("p (h d) -> p h d", d=DH),
            )

        def emit_proj_one(tb1, use_act=False):
            # output projection for one 128-token block.
            pp = sps.tile([P, 1024], F32, name="pp", tag="sp")
            for pr in range(2):
                for cb in range(2):
                    nc.tensor.matmul(
                        pp[:, cb * 512 : (cb + 1) * 512],
                        oT[pr][:, tb1 * P : (tb1 + 1) * P],
                        wo_sb[:, pr, cb * 512 : (cb + 1) * 512],
                        start=(pr == 0),
                        stop=(pr == 1),
                    )
            ot = osb.tile([P, 1024], F16, name="ot")
            if use_act:
                nc.scalar.copy(ot[:, 0:512], pp[:, 0:512])
            else:
                nc.vector.tensor_copy(ot[:, 0:512], pp[:, 0:512])
            nc.vector.tensor_copy(ot[:, 512:1024], pp[:, 512:1024])
            nc.sync.dma_start(out[tb1 * P : (tb1 + 1) * P, :], ot)

        def attn_block(qb, pr, fillers=(), fast_norm=False):
            """S^T blocks [k-tile(128) x q-block(<=512)]; 2 heads packed on
            PE row groups (K=64 each); exp via ScalarE; PV lagged 2 steps;
            `fillers` (thunks of ~0.8-1.8us PE work) are drained evenly
            across the j loop to keep the PE dense while exp paces."""
            fillers = list(fillers)
            op = ops.tile([P, 1024], F32, name="op", tag="op")
            nk = 4 * qb + 4

            def geom(j):
                r = j - 4 * qb
                width = 512 - r * P if r >= 0 else 512
                col0 = r * P if r >= 0 else 0
                return r, width, col0

            def emit_o(j, pts):
                _, width, col0 = geom(j)
                pt = pts.pop(j)
                for h in range(2):
                    nc.tensor.matmul(
                        op[0 : DH + 1, h * 512 + col0 : (h + 1) * 512],
                        vS[:, j, pr * 2 + h, :],
                        pt[:, h * 512 : h * 512 + width],
                        start=(j == 0),
                        stop=(j == nk - 1),
                        skip_group_check=True,
                    )

            pts = {}
            nf = len(fillers)
            fdone = 0
            for j in range(nk):
                r, width, col0 = geom(j)
                qoff = qb * 512 + col0
                sp_ = sps.tile([P, 1024], F32, name="sp_", tag="sp")
                for h in range(2):
                    nc.tensor.matmul(
                        sp_[:, h * 512 : h * 512 + width],
                        kT[pr][h * DH : (h + 1) * DH, j * P : (j + 1) * P],
                        qT[pr][h * DH : (h + 1) * DH, qoff : qoff + width],
                        start=True,
                        stop=True,
                    )
                pt = ptp.tile([P, 1024], F16, name="pt")
                s3 = sp_.rearrange("p (h w) -> p h w", h=2)[:, :, 0:width]
                p3 = pt.rearrange("p (h w) -> p h w", h=2)[:, :, 0:width]
                nc.scalar.activation(p3, s3, EXP, scale=SCALE)
                if r >= 0:
                    # triangular mask on the first 128 valid columns
                    for h in range(2):
                        nc.vector.tensor_mul(
                            pt[:, h * 512 : h * 512 + P],
                            pt[:, h * 512 : h * 512 + P],
                            tri_sb,
                        )
                pts[j] = pt
                # spread fillers evenly over the j loop
                want = nf * (j + 1) // nk
                while fdone < want:
                    fillers[fdone]()
                    fdone += 1
                # software pipeline: O' lagged two steps behind S/exp
                if j > 1:
                    emit_o(j - 2, pts)
            if nk > 1:
                emit_o(nk - 2, pts)
            emit_o(nk - 1, pts)
            while fdone < nf:
                fillers[fdone]()
                fdone += 1

            # fast unnormalized evacuation frees the op PSUM slot; the
            # reciprocal/broadcast chain then runs off the critical engine.
            qs = slice(qb * 512, (qb + 1) * 512)
            nc.vector.tensor_copy(
                oTu[pr][:, :, qs],
                op.rearrange("p (h w) -> p h w", h=2)[0 : DH + 1],
            )
            if fast_norm:
                # latency-optimized normalize for the final section: the
                # denominator row is broadcast down 64 partitions with a
                # K=1 matmul instead of the DRAM round trip (saves ~8us
                # of serial DMA latency on the critical tail and keeps
                # the PE warm through the endgame).
                bcp = sps.tile([P, 1024], F32, name="bcp", tag="sp")
                for h in range(2):
                    nc.tensor.matmul(
                        bcp[0:DH, h * 512 : (h + 1) * 512],
                        ones_bc[DH : DH + 1, :],
                        oTu[pr][DH : DH + 1, h, qs],
                        start=True,
                        stop=True,
                    )
                rec = nrm.tile([DH, 1024], F32, name="rec", tag="bc")
                nc.vector.reciprocal(rec, bcp[0:DH])
                nc.vector.tensor_mul(
                    oT[pr][0:DH, qs], oTu[pr][0:DH, 0, qs], rec[:, 0:512]
                )
                o1f = nrm.tile([DH, 512], F16, name="o1f", tag="o1")
                nc.vector.tensor_mul(o1f, oTu[pr][0:DH, 1, qs], rec[:, 512:1024])
                nc.sync.dma_start(oT[pr][DH : 2 * DH, qs], o1f)
                return
            dd = dsc.tile([1024], F32, name="dd", tag="dd")
            nc.sync.dma_start(
                dd.rearrange("(h w) -> h w", h=2)[None],
                oTu[pr][DH : DH + 1, :, qs],
            )
            rsh = nrm.tile([P, 8], F32, name="rsh", tag="rsh")
            nc.sync.dma_start(rsh, dd.rearrange("(p c) -> p c", p=P))
            rr = nrm.tile([P, 8], F32, name="rr", tag="rr")
            nc.vector.reciprocal(rr, rsh)
            dd2 = dsc.tile([1024], F32, name="dd2", tag="dd2")
            nc.sync.dma_start(dd2.rearrange("(p c) -> p c", p=P), rr)
            bc = nrm.tile([DH, 1024], F32, name="bc", tag="bc")
            nc.sync.dma_start(
                bc[:, 0:512], dd2[None, 0:512].to_broadcast([DH, 512])
            )
            nc.sync.dma_start(
                bc[:, 512:1024], dd2[None, 512:1024].to_broadcast([DH, 512])
            )
            nc.vector.tensor_mul(
                oT[pr][0:DH, qs], oTu[pr][0:DH, 0, qs], bc[:, 0:512]
            )
            o1 = nrm.tile([DH, 512], F16, name="o1", tag="o1")
            nc.vector.tensor_mul(o1, oTu[pr][0:DH, 1, qs], bc[:, 512:1024])
            nc.sync.dma_start(oT[pr][DH : 2 * DH, qs], o1)

        # ---------- schedule ----------
        # Leading q/k chains for block 0 (scalar-idle warm-up region),
        # then attention sections with fillers drawn from later blocks'
        # projections and earlier blocks' output projections.  v-blocks
        # ride as fillers inside the section that first consumes them.
        for pr in range(2):
            qk_chain(0, pr, wq_sb, qT)
            qk_chain(0, pr, wk_sb, kT)

        def F(fn, *a, **kw):
            return lambda: fn(*a, **kw)

        attn_block(0, 0, [F(v_sub, 0, 0), F(v_sub, 0, 1), F(v_sub, 0, 2),
                          F(v_sub, 0, 3)])
        attn_block(0, 1, [F(qk_chain, 1, 0, wq_sb, qT), F(qk_chain, 1, 0, wk_sb, kT)])
        attn_block(1, 0, [F(qk_chain, 1, 1, wq_sb, qT), F(qk_chain, 1, 1, wk_sb, kT),
                          F(v_sub, 1, 0), F(v_sub, 1, 1), F(v_sub, 1, 2),
                          F(v_sub, 1, 3)])
        attn_block(1, 1, [F(qk_chain, 2, 0, wq_sb, qT), F(qk_chain, 2, 0, wk_sb, kT),
                          F(emit_proj_one, 0), F(emit_proj_one, 1)])
        attn_block(2, 0, [F(qk_chain, 2, 1, wq_sb, qT), F(qk_chain, 2, 1, wk_sb, kT),
                          F(v_sub, 2, 0), F(v_sub, 2, 1), F(v_sub, 2, 2),
                          F(v_sub, 2, 3), F(emit_proj_one, 2)])
        attn_block(2, 1, [F(qk_chain, 3, 0, wq_sb, qT), F(qk_chain, 3, 0, wk_sb, kT),
                          F(qk_chain, 3, 1, wq_sb, qT), F(qk_chain, 3, 1, wk_sb, kT),
                          F(emit_proj_one, 3)])
        attn_block(3, 0, [F(v_sub, 3, 0), F(v_sub, 3, 1), F(v_sub, 3, 2), F(v_sub, 3, 3),
                          F(emit_proj_one, 4), F(emit_proj_one, 5),
                          F(emit_proj_one, 6), F(emit_proj_one, 7)])
        attn_block(3, 1, [F(emit_proj_one, 8), F(emit_proj_one, 9),
                          F(emit_proj_one, 10), F(emit_proj_one, 11)],
                   fast_norm=True)
        for tb1 in range(12, 16):
            emit_proj_one(tb1, use_act=True)


def build_bass():
    nc = bacc.Bacc("TRN2", target_bir_lowering=False, debug=False, num_devices=8)
    xT = nc.dram_tensor("xT", [P, NQB, KT, 512], F16, kind="ExternalInput").ap()
    wq = nc.dram_tensor("wq", [P, 2, KT, P], F16, kind="ExternalInput").ap()
    wk = nc.dram_tensor("wk", [P, 2, KT, P], F16, kind="ExternalInput").ap()
    wv = nc.dram_tensor("wv", [P, KT, 2 * P], F16, kind="ExternalInput").ap()
    wo = nc.dram_tensor("wo", [P, 2, C], F16, kind="ExternalInput").ap()
    tri = nc.dram_tensor("tri", [P, P], F16, kind="ExternalInput").ap()
    vones = nc.dram_tensor(
        "vones", [P, NKT, HPC, 1], F16, kind="ExternalInput"
    ).ap()
    out = nc.dram_tensor("out", [T, C], F16, kind="ExternalOutput").ap()
    with tile.TileContext(nc) as tc:
        _body(tc, nc, xT, wq, wk, wv, wo, tri, vones, out)
    nc.compile()
    return nc


def make_in_maps(x, w_qkv, w_out):
    """Host-side sharding: returns the 8 per-core input dicts."""
    x = np.ascontiguousarray(np.asarray(x, dtype=np.float32))
    w_qkv = np.ascontiguousarray(np.asarray(w_qkv, dtype=np.float32))
    w_out = np.ascontiguousarray(np.asarray(w_out, dtype=np.float32))
    kk = np.arange(P)
    tri = (kk[None, :] >= kk[:, None]).astype(np.float16)  # [k, q]: q >= k

    def pack_w_pr(w):  # [C, 2P] -> [P, 2, KT, P], pr-major contiguous halves
        return np.ascontiguousarray(
            w.astype(np.float16).reshape(KT, P, 2, P).transpose(1, 2, 0, 3)
        )

    def pack_w(w):  # [C, n] -> [P, KT, n], per-partition contiguous
        n = w.shape[1]
        return np.ascontiguousarray(
            w.astype(np.float16).reshape(KT, P, n).transpose(1, 0, 2)
        )

    # x[b].T [C, T] -> [P, block, kt, 512], fully contiguous per partition
    xTb = [
        np.ascontiguousarray(
            x[b].T.astype(np.float16)
            .reshape(KT, P, NQB, 512)
            .transpose(1, 2, 0, 3)
        )
        for b in range(B)
    ]
    in_maps = []
    for c in range(8):
        b = c // 4
        g = c % 4
        h0 = HPC * g * DH  # 256*g
        wo_c = w_out[h0 : h0 + 2 * P, :].astype(np.float16)
        in_maps.append(
            {
                "xT": xTb[b],
                "wq": pack_w_pr(w_qkv[:, h0 : h0 + 2 * P]),
                "wk": pack_w_pr(w_qkv[:, C + h0 : C + h0 + 2 * P]),
                "wv": pack_w(w_qkv[:, 2 * C + h0 : 2 * C + h0 + 2 * P]),
                "wo": np.ascontiguousarray(
                    wo_c.reshape(2, P, C).transpose(1, 0, 2)
                ),
                "tri": np.ascontiguousarray(tri),
                "vones": np.ones((P, NKT, HPC, 1), dtype=np.float16),
            }
        )
    return in_maps


_NC_CACHE = None
LAST_RESULTS = None  # BassKernelResults of the most recent run (for profiling)
TRACE = False


def kernel(x, w_qkv, w_out):
    global _NC_CACHE, LAST_RESULTS
    if _NC_CACHE is None:
        _NC_CACHE = build_bass()
    nc = _NC_CACHE
    in_maps = make_in_maps(x, w_qkv, w_out)
    res = bass_utils.run_bass_kernel_spmd(
        nc, in_maps, core_ids=list(range(8)), trace=TRACE
    )
    LAST_RESULTS = res
    partials = [res.results[c]["out"] for c in range(8)]
    out = np.zeros((B, T, C), dtype=np.float32)
    for c in range(8):
        out[c // 4] += partials[c].astype(np.float32)
    return out


if __name__ == "__main__":
    # smoke test with random data
    rng = np.random.default_rng(0)
    x = rng.standard_normal((B, T, C), dtype=np.float32)
    w_qkv = rng.standard_normal((C, 3 * C), dtype=np.float32) / np.sqrt(C)
    w_out = rng.standard_normal((C, C), dtype=np.float32) / np.sqrt(C)
    o = kernel(x, w_qkv, w_out)
    print(o.shape, o.dtype)


# revision 24
# speedup vs baseline: 1.1575x; 1.0557x over previous
"""Trainium2 Bass kernel for causal multi-head attention block.

Reference computation (fp32):
    qkv = x @ w_qkv;  q,k,v = split(qkv)
    attn = softmax(causal_mask(q k^T / sqrt(64)))
    out  = (attn @ v reassembled) @ w_out

Sharding over 8 NeuronCores: core c handles batch b = c//4 and heads
4*(c%4) .. 4*(c%4)+3 (4 of 16 heads).  Each core computes the rank-256
partial product of the output projection restricted to its heads'
channels; the host sums the 4 partials per batch.

Everything streams and computes in fp16 (fp32 PSUM accumulation).
The warm-up matmul burst and all constant tiles live in pools that stay
allocated for the whole kernel, so no SBUF address is reused between the
warm-up and the weight tiles -- input DMAs start at ~6.5us (right after
the NEFF preamble) instead of waiting on the warm-up semaphore.

Schedule: a single software-pipelined stream.  Each attention section
(qb, pr) runs the baseline lag-2 S/exp/PV pipeline and additionally
interleaves "filler" chunks (q/k projection chains of later blocks, v
blocks, output-projection tiles of earlier blocks) into its j loop so
the PE stays dense while the ScalarE exp stream paces ~0.6us/j behind.
"""

import sys

for _p in ("/opt/trn_rl_repo", "/root/.axon_site/_ro/trn_rl_repo"):
    if _p not in sys.path:
        sys.path.append(_p)

import numpy as np

import concourse.bass as bass
import concourse.mybir as mybir
import concourse.tile as tile
from concourse import bacc, bass_utils

P = 128
B, T, C = 2, 2048, 1024
HPC = 4            # heads per core
DH = 64            # head dim
KT = C // P        # 8 contraction tiles over d_model
NQB = T // 512     # 4 query blocks of 512
NKT = T // P       # 16 key tiles of 128
F32 = mybir.dt.float32
R32 = mybir.dt.float32r
F16 = mybir.dt.float16
EXP = mybir.ActivationFunctionType.Exp
SCALE = 1.0 / 8.0  # 1/sqrt(DH)
# >= 3.4us of back-to-back matmuls at the cold 1.2 GHz clock (53ns per
# N=64 matmul), so the HAM activity window flips to K=8/8 before the
# first projection chains issue.  48 was not enough (2.5us) and the
# whole first phase ran at half clock.
NWARM = 96
RECIP = mybir.ActivationFunctionType.Reciprocal


def _body(tc, nc, xT, wq, wk, wv, wo, tri, vones, out):
    with (
        tc.tile_pool(name="const", bufs=1) as cpool,
        tc.tile_pool(name="xt", bufs=1) as xpool,
        tc.tile_pool(name="psA", bufs=2, space="PSUM") as psA,
        tc.tile_pool(name="sps", bufs=2, space="PSUM") as sps,
        tc.tile_pool(name="ops", bufs=1, space="PSUM") as ops,
        tc.tile_pool(name="ptp", bufs=8) as ptp,
        tc.tile_pool(name="nrm", bufs=2) as nrm,
        tc.tile_pool(name="dsc", bufs=2, space="DRAM") as dsc,
        tc.tile_pool(name="osb", bufs=3) as osb,
    ):
        # ---- PE warm-up: releases the HAM clock gate (1.2 -> 2.4 GHz)
        # during the NEFF preamble + input-DMA window.  The source tile
        # lives in the persistent const pool and the PSUM target shares
        # the "op" ring slot, so no SBUF/PSUM address is ever handed back
        # for reuse -- the input DMAs don't wait on the warm-up.
        wsrc = cpool.tile([P, 64], F16, name="wsrc")
        nc.vector.memset(wsrc, 0.0)

        def warm_burst(n):
            wdst = ops.tile([64, 64], F32, name="wdst", tag="op")
            for i in range(n):
                nc.tensor.matmul(
                    wdst, wsrc, wsrc, start=(i == 0), stop=(i == n - 1)
                )

        warm_burst(NWARM)

        # ---- constants.  wq/wk are packed pr-major so each 256KB half is
        # one contiguous DMA and the first q/k chains only wait on their
        # own half.
        wq_sb = cpool.tile([P, 2, KT, P], F16, name="wq_sb")
        wk_sb = cpool.tile([P, 2, KT, P], F16, name="wk_sb")
        wv_sb = cpool.tile([P, KT, 2 * P], F16, name="wv_sb")
        wo_sb = cpool.tile([P, 2, C], F16, name="wo_sb")
        tri_sb = cpool.tile([P, P], F16, name="tri_sb")
        nc.scalar.dma_start(wq_sb[:, 0], wq[:, 0])
        nc.scalar.dma_start(wk_sb[:, 0], wk[:, 0])
        nc.scalar.dma_start(wq_sb[:, 1], wq[:, 1])
        nc.scalar.dma_start(wk_sb[:, 1], wk[:, 1])
        # tri (needed by the first attention section) and vones (first PV)
        # go ahead of the bulky wo (first needed by proj at ~60us) so the
        # SWDGE queue doesn't crowd the critical startup window.
        nc.gpsimd.dma_start(tri_sb, tri)

        # preload the exp ACT table set during the startup DMA window
        warm = cpool.tile([1, 2], F32, name="warm")
        nc.vector.memset(warm, 0.0)
        nc.scalar.activation(warm, warm, EXP, scale=1.0)

        # all-ones stationary column used to broadcast the softmax
        # denominators across partitions with a K=1 matmul (fast-path
        # normalize for the last attention section).
        ones_bc = cpool.tile([P, DH], F32, name="ones_bc")
        nc.vector.memset(ones_bc, 1.0)

        # persistent stores
        qT = [cpool.tile([P, T], F16, name=f"qT{pr}") for pr in range(2)]
        kT = [cpool.tile([P, T], F16, name=f"kT{pr}") for pr in range(2)]
        # v with a fused ones column: [T-part, ktile, head, 65]
        vS = cpool.tile([P, NKT, HPC, DH + 1], F16, name="vS")
        nc.gpsimd.dma_start(vS[:, :, :, DH : DH + 1], vones)
        nc.gpsimd.dma_start(wo_sb, wo)
        oT = [cpool.tile([P, T], F16, name=f"oT{pr}") for pr in range(2)]
        oTu = [cpool.tile([DH + 1, 2, T], F32, name=f"oTu{pr}") for pr in range(2)]

        xts = [
            xpool.tile([P, KT, 512], F16, name=f"xt{i}", bufs=1)
            for i in range(NQB)
        ]
        for qu in range(4):
            nc.sync.dma_start(
                xts[0][:, 2 * qu : 2 * qu + 2, :], xT[:, 0, 2 * qu : 2 * qu + 2]
            )
        nc.sync.dma_start(wv_sb, wv)
        for later in range(1, NQB):
            nc.sync.dma_start(xts[later], xT[:, later])

        # ---------- building blocks ----------
        def qk_chain(tb5, pr, w_sb, dst):
            xt = xts[tb5]
            qp = psA.tile([P, 512], F32, name="qp", tag="qkv")
            for kt in range(KT):
                nc.tensor.matmul(
                    qp,
                    w_sb[:, pr, kt, :],
                    xt[:, kt, :],
                    start=(kt == 0),
                    stop=(kt == KT - 1),
                )
            nc.vector.tensor_copy(dst[pr][:, tb5 * 512 : (tb5 + 1) * 512], qp)

        def v_sub(tb5, sub):
            xt = xts[tb5]
            tb1 = tb5 * 4 + sub
            vp = psA.tile([P, 512], F32, name="vp", tag="qkv")
            for kt in range(KT):
                nc.tensor.matmul(
                    vp[:, 0:256],
                    xt[:, kt, sub * P : (sub + 1) * P],
                    wv_sb[:, kt, :],
                    start=(kt == 0),
                    stop=(kt == KT - 1),
                )
            nc.vector.tensor_copy(
                vS[:, tb1, :, 0:DH],
                vp[:, 0:256].rearrange("p (h d) -> p h d", d=DH),
            )

        def emit_proj_one(tb1, use_act=False):
            # output projection for one 128-token block.
            pp = sps.tile([P, 1024], F32, name="pp", tag="sp")
            for pr in range(2):
                for cb in range(2):
                    nc.tensor.matmul(
                        pp[:, cb * 512 : (cb + 1) * 512],
                        oT[pr][:, tb1 * P : (tb1 + 1) * P],
                        wo_sb[:, pr, cb * 512 : (cb + 1) * 512],
                        start=(pr == 0),
                        stop=(pr == 1),
                    )
            ot = osb.tile([P, 1024], F16, name="ot")
            if use_act:
                nc.scalar.copy(ot[:, 0:512], pp[:, 0:512])
            else:
                nc.vector.tensor_copy(ot[:, 0:512], pp[:, 0:512])
            nc.sync.dma_start(out[tb1 * P : (tb1 + 1) * P, 0:512], ot[:, 0:512])
            nc.vector.tensor_copy(ot[:, 512:1024], pp[:, 512:1024])
            nc.sync.dma_start(
                out[tb1 * P : (tb1 + 1) * P, 512:1024], ot[:, 512:1024]
            )

        def attn_block(qb, pr, fillers=(), fast_norm=False):
            """S^T blocks [k-tile(128) x q-block(<=512)]; 2 heads packed on
            PE row groups (K=64 each); exp via ScalarE; PV lagged 2 steps;
            `fillers` (thunks of ~0.8-1.8us PE work) are drained evenly
            across the j loop to keep the PE dense while exp paces."""
            fillers = list(fillers)
            op = ops.tile([P, 1024], F32, name="op", tag="op")
            nk = 4 * qb + 4

            def geom(j):
                r = j - 4 * qb
                width = 512 - r * P if r >= 0 else 512
                col0 = r * P if r >= 0 else 0
                return r, width, col0

            def emit_o(j, pts):
                _, width, col0 = geom(j)
                pt = pts.pop(j)
                for h in range(2):
                    nc.tensor.matmul(
                        op[0 : DH + 1, h * 512 + col0 : (h + 1) * 512],
                        vS[:, j, pr * 2 + h, :],
                        pt[:, h * 512 : h * 512 + width],
                        start=(j == 0),
                        stop=(j == nk - 1),
                        skip_group_check=True,
                    )

            pts = {}
            nf = len(fillers)
            fdone = 0
            for j in range(nk):
                r, width, col0 = geom(j)
                qoff = qb * 512 + col0
                sp_ = sps.tile([P, 1024], F32, name="sp_", tag="sp")
                for h in range(2):
                    nc.tensor.matmul(
                        sp_[:, h * 512 : h * 512 + width],
                        kT[pr][h * DH : (h + 1) * DH, j * P : (j + 1) * P],
                        qT[pr][h * DH : (h + 1) * DH, qoff : qoff + width],
                        start=True,
                        stop=True,
                    )
                pt = ptp.tile([P, 1024], F16, name="pt")
                s3 = sp_.rearrange("p (h w) -> p h w", h=2)[:, :, 0:width]
                p3 = pt.rearrange("p (h w) -> p h w", h=2)[:, :, 0:width]
                nc.scalar.activation(p3, s3, EXP, scale=SCALE)
                if r >= 0:
                    # triangular mask on the first 128 valid columns
                    for h in range(2):
                        nc.vector.tensor_mul(
                            pt[:, h * 512 : h * 512 + P],
                            pt[:, h * 512 : h * 512 + P],
                            tri_sb,
                        )
                pts[j] = pt
                # spread fillers evenly over the j loop
                want = nf * (j + 1) // nk
                while fdone < want:
                    fillers[fdone]()
                    fdone += 1
                # software pipeline: O' lagged two steps behind S/exp
                if j > 1:
                    emit_o(j - 2, pts)
            if nk > 1:
                emit_o(nk - 2, pts)
            emit_o(nk - 1, pts)
            while fdone < nf:
                fillers[fdone]()
                fdone += 1

            # fast unnormalized evacuation frees the op PSUM slot; the
            # reciprocal/broadcast chain then runs off the critical engine.
            # The copy goes through ScalarE (which has ~50us of slack and
            # is idle at each section end) so the op PSUM ring frees
            # without queueing behind the busy DVE.
            qs = slice(qb * 512, (qb + 1) * 512)
            nc.scalar.copy(
                oTu[pr][:, :, qs],
                op.rearrange("p (h w) -> p h w", h=2)[0 : DH + 1],
            )
            if fast_norm:
                # latency-optimized normalize for the final section: the
                # denominator row is broadcast down 64 partitions with a
                # K=1 matmul instead of the DRAM round trip, and the
                # reciprocal uses the ~5x-faster approx custom-DVE op
                # (51 ULP -- far below the fp16 noise floor); the exact
                # DVE reciprocal is ~8 cycles/element and measured 6.5us
                # on [64,1024].
                bcp = sps.tile([P, 1024], F32, name="bcp", tag="sp")
                for h in range(2):
                    nc.tensor.matmul(
                        bcp[0:DH, h * 512 : (h + 1) * 512],
                        ones_bc[DH : DH + 1, :],
                        oTu[pr][DH : DH + 1, h, qs],
                        start=True,
                        stop=True,
                    )
                rec = nrm.tile([DH, 1024], F32, name="rec", tag="bc")
                nc.vector.reciprocal_approx_fast(rec, bcp[0:DH])
                nc.vector.tensor_mul(
                    oT[pr][0:DH, qs], oTu[pr][0:DH, 0, qs], rec[:, 0:512]
                )
                o1f = nrm.tile([DH, 512], F16, name="o1f", tag="o1")
                nc.vector.tensor_mul(o1f, oTu[pr][0:DH, 1, qs], rec[:, 512:1024])
                nc.sync.dma_start(oT[pr][DH : 2 * DH, qs], o1f)
                return
            dd = dsc.tile([1024], F32, name="dd", tag="dd")
            nc.sync.dma_start(
                dd.rearrange("(h w) -> h w", h=2)[None],
                oTu[pr][DH : DH + 1, :, qs],
            )
            rsh = nrm.tile([P, 8], F32, name="rsh", tag="rsh")
            nc.sync.dma_start(rsh, dd.rearrange("(p c) -> p c", p=P))
            rr = nrm.tile([P, 8], F32, name="rr", tag="rr")
            nc.vector.reciprocal(rr, rsh)
            dd2 = dsc.tile([1024], F32, name="dd2", tag="dd2")
            nc.sync.dma_start(dd2.rearrange("(p c) -> p c", p=P), rr)
            bc = nrm.tile([DH, 1024], F32, name="bc", tag="bc")
            nc.sync.dma_start(
                bc[:, 0:512], dd2[None, 0:512].to_broadcast([DH, 512])
            )
            nc.sync.dma_start(
                bc[:, 512:1024], dd2[None, 512:1024].to_broadcast([DH, 512])
            )
            nc.vector.tensor_mul(
                oT[pr][0:DH, qs], oTu[pr][0:DH, 0, qs], bc[:, 0:512]
            )
            o1 = nrm.tile([DH, 512], F16, name="o1", tag="o1")
            nc.vector.tensor_mul(o1, oTu[pr][0:DH, 1, qs], bc[:, 512:1024])
            nc.sync.dma_start(oT[pr][DH : 2 * DH, qs], o1)

        # ---------- schedule ----------
        # Leading q/k chains for block 0 (scalar-idle warm-up region),
        # then attention sections with fillers drawn from later blocks'
        # projections and earlier blocks' output projections.  v-blocks
        # ride as fillers inside the section that first consumes them.
        # Short warm bursts between the first chains keep the PE's HAM
        # activity window busy across the input-DMA arrival gaps, so the
        # clock never drops back to 1.2 GHz mid-startup.
        qk_chain(0, 0, wq_sb, qT)
        warm_burst(24)
        qk_chain(0, 0, wk_sb, kT)
        warm_burst(24)
        qk_chain(0, 1, wq_sb, qT)
        warm_burst(24)
        qk_chain(0, 1, wk_sb, kT)

        def F(fn, *a, **kw):
            return lambda: fn(*a, **kw)

        attn_block(0, 0, [F(v_sub, 0, 0), F(v_sub, 0, 1), F(v_sub, 0, 2),
                          F(v_sub, 0, 3)])
        attn_block(0, 1, [F(qk_chain, 1, 0, wq_sb, qT), F(qk_chain, 1, 0, wk_sb, kT)])
        attn_block(1, 0, [F(qk_chain, 1, 1, wq_sb, qT), F(qk_chain, 1, 1, wk_sb, kT),
                          F(v_sub, 1, 0), F(v_sub, 1, 1), F(v_sub, 1, 2),
                          F(v_sub, 1, 3)])
        attn_block(1, 1, [F(qk_chain, 2, 0, wq_sb, qT), F(qk_chain, 2, 0, wk_sb, kT),
                          F(emit_proj_one, 0), F(emit_proj_one, 1)])
        attn_block(2, 0, [F(qk_chain, 2, 1, wq_sb, qT), F(qk_chain, 2, 1, wk_sb, kT),
                          F(v_sub, 2, 0), F(v_sub, 2, 1), F(v_sub, 2, 2),
                          F(v_sub, 2, 3), F(emit_proj_one, 2)])
        attn_block(2, 1, [F(qk_chain, 3, 0, wq_sb, qT), F(qk_chain, 3, 0, wk_sb, kT),
                          F(qk_chain, 3, 1, wq_sb, qT), F(qk_chain, 3, 1, wk_sb, kT),
                          F(emit_proj_one, 3)])
        attn_block(3, 0, [F(v_sub, 3, 0), F(v_sub, 3, 1), F(v_sub, 3, 2), F(v_sub, 3, 3),
                          F(emit_proj_one, 4), F(emit_proj_one, 5),
                          F(emit_proj_one, 6), F(emit_proj_one, 7)])
        attn_block(3, 1, [F(emit_proj_one, 8), F(emit_proj_one, 9),
                          F(emit_proj_one, 10), F(emit_proj_one, 11)],
                   fast_norm=True)
        for tb1 in range(12, 16):
            emit_proj_one(tb1, use_act=True)


def build_bass():
    nc = bacc.Bacc("TRN2", target_bir_lowering=False, debug=False, num_devices=8)
    xT = nc.dram_tensor("xT", [P, NQB, KT, 512], F16, kind="ExternalInput").ap()
    wq = nc.dram_tensor("wq", [P, 2, KT, P], F16, kind="ExternalInput").ap()
    wk = nc.dram_tensor("wk", [P, 2, KT, P], F16, kind="ExternalInput").ap()
    wv = nc.dram_tensor("wv", [P, KT, 2 * P], F16, kind="ExternalInput").ap()
    wo = nc.dram_tensor("wo", [P, 2, C], F16, kind="ExternalInput").ap()
    tri = nc.dram_tensor("tri", [P, P], F16, kind="ExternalInput").ap()
    vones = nc.dram_tensor(
        "vones", [P, NKT, HPC, 1], F16, kind="ExternalInput"
    ).ap()
    out = nc.dram_tensor("out", [T, C], F16, kind="ExternalOutput").ap()
    with tile.TileContext(nc) as tc:
        _body(tc, nc, xT, wq, wk, wv, wo, tri, vones, out)
    nc.compile()
    return nc


def make_in_maps(x, w_qkv, w_out):
    """Host-side sharding: returns the 8 per-core input dicts."""
    x = np.ascontiguousarray(np.asarray(x, dtype=np.float32))
    w_qkv = np.ascontiguousarray(np.asarray(w_qkv, dtype=np.float32))
    w_out = np.ascontiguousarray(np.asarray(w_out, dtype=np.float32))
    kk = np.arange(P)
    tri = (kk[None, :] >= kk[:, None]).astype(np.float16)  # [k, q]: q >= k

    def pack_w_pr(w):  # [C, 2P] -> [P, 2, KT, P], pr-major contiguous halves
        return np.ascontiguousarray(
            w.astype(np.float16).reshape(KT, P, 2, P).transpose(1, 2, 0, 3)
        )

    def pack_w(w):  # [C, n] -> [P, KT, n], per-partition contiguous
        n = w.shape[1]
        return np.ascontiguousarray(
            w.astype(np.float16).reshape(KT, P, n).transpose(1, 0, 2)
        )

    # x[b].T [C, T] -> [P, block, kt, 512], fully contiguous per partition
    xTb = [
        np.ascontiguousarray(
            x[b].T.astype(np.float16)
            .reshape(KT, P, NQB, 512)
            .transpose(1, 2, 0, 3)
        )
        for b in range(B)
    ]
    in_maps = []
    for c in range(8):
        b = c // 4
        g = c % 4
        h0 = HPC * g * DH  # 256*g
        wo_c = w_out[h0 : h0 + 2 * P, :].astype(np.float16)
        in_maps.append(
            {
                "xT": xTb[b],
                "wq": pack_w_pr(w_qkv[:, h0 : h0 + 2 * P]),
                "wk": pack_w_pr(w_qkv[:, C + h0 : C + h0 + 2 * P]),
                "wv": pack_w(w_qkv[:, 2 * C + h0 : 2 * C + h0 + 2 * P]),
                "wo": np.ascontiguousarray(
                    wo_c.reshape(2, P, C).transpose(1, 0, 2)
                ),
                "tri": np.ascontiguousarray(tri),
                "vones": np.ones((P, NKT, HPC, 1), dtype=np.float16),
            }
        )
    return in_maps


_NC_CACHE = None
LAST_RESULTS = None  # BassKernelResults of the most recent run (for profiling)
TRACE = False


def kernel(x, w_qkv, w_out):
    global _NC_CACHE, LAST_RESULTS
    if _NC_CACHE is None:
        _NC_CACHE = build_bass()
    nc = _NC_CACHE
    in_maps = make_in_maps(x, w_qkv, w_out)
    res = bass_utils.run_bass_kernel_spmd(
        nc, in_maps, core_ids=list(range(8)), trace=TRACE
    )
    LAST_RESULTS = res
    partials = [res.results[c]["out"] for c in range(8)]
    out = np.zeros((B, T, C), dtype=np.float32)
    for c in range(8):
        out[c // 4] += partials[c].astype(np.float32)
    return out


if __name__ == "__main__":
    # smoke test with random data
    rng = np.random.default_rng(0)
    x = rng.standard_normal((B, T, C), dtype=np.float32)
    w_qkv = rng.standard_normal((C, 3 * C), dtype=np.float32) / np.sqrt(C)
    w_out = rng.standard_normal((C, C), dtype=np.float32) / np.sqrt(C)
    o = kernel(x, w_qkv, w_out)
    print(o.shape, o.dtype)
